# revision 7
# baseline (speedup 1.0000x reference)
"""BitNet-style quantized linear on 8 trn2 cores.

out = act_quant(rms_norm(x)) @ weight_quant(w).T

Sharding: tokens x2 (r), out_features x4 (c).  Each core:
  x shard  [4096, 2048] f32, w shard [2048, 2048] f32 -> out [4096, 2048] f32.
Weight abs-mean scale is global: exact coarse/fine split accumulation
on-device + 8-core AllReduce so the fp32 mean bit-matches the reference's
(verified for the fixed seed; ternary pattern then matches exactly).

Matmul runs as exact integer arithmetic in bf16 (q in [-127,127], ternary
weights), accumulated in fp32 PSUM, then scaled by per-token 1/(s_t*ws).
"""

import sys

for p in ("/opt/trn_rl_repo",):
    if p not in sys.path:
        sys.path.insert(0, p)

import numpy as np

B, S, DIN, DOUT = 4, 2048, 2048, 8192
NTOK = B * S
NCORES = 8
R_TOK, C_OUT = 2, 4
TOK_LOC = NTOK // R_TOK      # 4096
O_LOC = DOUT // C_OUT        # 2048
KT = DIN // 128              # 16 k-tiles
TB = TOK_LOC // 128          # 32 token blocks
WT = O_LOC // 128            # 16 weight tiles
OC = O_LOC // 512            # 4 out chunks

MROUND = 12582912.0          # 3 * 2^22: (x + M) - M == rint(x) for |x| < 2^22
EPS = float(np.finfo(np.float32).eps)
INV_CNT = 1.0 / (2 * DOUT * DIN)   # allreduce double-counts w (x2 token replicas); 2^-25 exact
F32MAX = 3.4028235e38


def build_nc():
    import concourse.bass as bass
    import concourse.tile as tile
    from concourse import bacc, mybir
    from concourse import bass_isa
    from concourse.masks import make_identity

    f32 = mybir.dt.float32
    bf16 = mybir.dt.bfloat16

    nc = bacc.Bacc(None, target_bir_lowering=False, num_devices=NCORES)

    x_in = nc.dram_tensor("x", [TOK_LOC, DIN], f32, kind="ExternalInput")
    w_in = nc.dram_tensor("w", [O_LOC, DIN], f32, kind="ExternalInput")
    out_d = nc.dram_tensor("out", [TOK_LOC, O_LOC], f32, kind="ExternalOutput")

    with tile.TileContext(nc) as tc:
        with (
            tc.tile_pool(name="f32p", bufs=3) as f32p,          # [128,2048] f32 loads
            tc.tile_pool(name="qf", bufs=2) as qfp,             # [128,2048] f32 quant tmp
            tc.tile_pool(name="bfp", bufs=4) as bfp,            # [128,2048] bf16
            tc.tile_pool(name="qT", bufs=4) as qTp,             # [128,16,128] bf16
            tc.tile_pool(name="outp", bufs=2) as outp,          # [128,2048] f32
            tc.tile_pool(name="pst", bufs=4, space="PSUM") as pst,     # transpose psum
            tc.tile_pool(name="psm", bufs=4, space="PSUM") as psm,     # matmul psum
            tc.tile_pool(name="sing", bufs=1) as sing,
            tc.tile_pool(name="tiv", bufs=4) as tivp,           # per-tb total_inv
            tc.tile_pool(name="dram", bufs=1, space="DRAM") as dram,
        ):
            ident = sing.tile([128, 128], bf16)
            make_identity(nc, ident)
            mconst = sing.tile([128, 1], f32)
            nc.vector.memset(mconst, MROUND)
            zconst = sing.tile([128, 1], f32)
            nc.vector.memset(zconst, 0.0)

            wT = sing.tile([128, KT, O_LOC], bf16)   # 8.4MB resident w^T ternary

            # ---------------- Phase W1: global |w| mean ----------------
            A = sing.tile([128, WT], f32)
            nc.vector.memset(A, 0.0)
            for wt in range(WT):
                wtile = f32p.tile([128, DIN], f32, tag="f32t")
                nc.sync.dma_start(out=wtile, in_=w_in[wt * 128:(wt + 1) * 128, :])
                cp = sing.tile([128, KT], f32, tag=f"cp{wt % 2}")
                nc.vector.tensor_reduce(
                    cp, wtile.rearrange("p (c k) -> p c k", k=128),
                    axis=mybir.AxisListType.X, op=mybir.AluOpType.add,
                    apply_absolute_value=True,
                )
                # A[:, wt] = sum of the 16 chunk sums of this tile
                nc.vector.tensor_reduce(
                    A[:, wt:wt + 1], cp, axis=mybir.AxisListType.X,
                    op=mybir.AluOpType.add,
                )
            # coarse/fine split: C = rint(A) (exact int sums), F = A - C
            Cc = sing.tile([128, WT], f32)
            Ff = sing.tile([128, WT], f32)
            nc.vector.tensor_scalar(Cc, A, MROUND, MROUND,
                                    mybir.AluOpType.add, mybir.AluOpType.subtract)
            nc.vector.tensor_tensor(out=Ff, in0=A, in1=Cc, op=mybir.AluOpType.subtract)
            CF = sing.tile([128, 2], f32)
            nc.vector.tensor_reduce(CF[:, 0:1], Cc, axis=mybir.AxisListType.X,
                                    op=mybir.AluOpType.add)
            nc.vector.tensor_reduce(CF[:, 1:2], Ff, axis=mybir.AxisListType.X,
                                    op=mybir.AluOpType.add)
            CFr = sing.tile([128, 2], f32)
            nc.gpsimd.partition_all_reduce(CFr, CF, channels=128,
                                           reduce_op=bass_isa.ReduceOp.add)
            # allreduce the two partials across the 8 cores
            z8 = sing.tile([1, 8], f32)
            nc.vector.memset(z8, 0.0)
            nc.vector.tensor_copy(z8[0:1, 0:2], CFr[0:1, 0:2])
            cc_in = dram.tile([1, 8], f32)
            cc_out = dram.tile([1, 8], f32)
            nc.sync.dma_start(out=cc_in, in_=z8)
            nc.gpsimd.collective_compute(
                "AllReduce", mybir.AluOpType.add,
                replica_groups=[list(range(NCORES))],
                ins=[cc_in.opt()], outs=[cc_out.opt()],
            )
            tot2 = sing.tile([128, 2], f32)
            nc.sync.dma_start(out=tot2, in_=cc_out[0:1, 0:2].to_broadcast([128, 2]))
            total = sing.tile([128, 1], f32)
            nc.vector.tensor_tensor(out=total, in0=tot2[:, 0:1], in1=tot2[:, 1:2],
                                    op=mybir.AluOpType.add)
            mean = sing.tile([128, 1], f32)
            nc.vector.tensor_scalar(mean, total, INV_CNT, 1e-5,
                                    mybir.AluOpType.mult, mybir.AluOpType.max)
            wsc = sing.tile([128, 1], f32)      # = 1/mean  (the reference's w scale)
            nc.vector.reciprocal(wsc, mean)
            inv_ws = sing.tile([128, 1], f32)   # = 1/wsc   (dequant factor)
            nc.vector.reciprocal(inv_ws, wsc)

            # ---------------- Phase W2: ternarize + transpose ----------------
            for wt in range(WT):
                wtile = f32p.tile([128, DIN], f32, tag="f32t")
                nc.sync.dma_start(out=wtile, in_=w_in[wt * 128:(wt + 1) * 128, :])
                u = qfp.tile([128, DIN], f32, tag="qf")
                nc.vector.tensor_scalar(u, wtile, wsc[:, 0:1], None,
                                        mybir.AluOpType.mult)
                t2 = qfp.tile([128, DIN], f32, tag="qf")
                nc.vector.tensor_scalar(t2, u, MROUND, MROUND + 1.0,
                                        mybir.AluOpType.add, mybir.AluOpType.min)
                tern = bfp.tile([128, DIN], bf16, tag="bf")
                nc.vector.tensor_scalar(tern, t2, MROUND - 1.0, MROUND,
                                        mybir.AluOpType.max, mybir.AluOpType.subtract)
                for k in range(KT):
                    ps = pst.tile([128, 128], bf16, tag="pst")
                    nc.tensor.transpose(ps, tern[:, k * 128:(k + 1) * 128], ident)
                    nc.vector.tensor_copy(wT[:, k, wt * 128:(wt + 1) * 128], ps)

            # ---------------- Main loop over token blocks ----------------
            for tb in range(TB):
                xt = f32p.tile([128, DIN], f32, tag="f32t")
                nc.sync.dma_start(out=xt, in_=x_in[tb * 128:(tb + 1) * 128, :])
                # stats
                amax = tivp.tile([128, 1], f32, tag="amax")
                nc.vector.tensor_reduce(amax, xt, axis=mybir.AxisListType.X,
                                        op=mybir.AluOpType.max,
                                        apply_absolute_value=True)
                sq = bfp.tile([128, DIN], bf16, tag="bf")
                ssq = tivp.tile([128, 1], f32, tag="ssq")
                nc.scalar.activation(sq, xt, mybir.ActivationFunctionType.Square,
                                     bias=zconst[:, 0:1], accum_out=ssq)
                ms = tivp.tile([128, 1], f32, tag="ms")
                nc.vector.tensor_scalar(ms, ssq, 1.0 / DIN, EPS,
                                        mybir.AluOpType.mult, mybir.AluOpType.add)
                rt = tivp.tile([128, 1], f32, tag="rt")
                nc.scalar.activation(rt, ms, mybir.ActivationFunctionType.Sqrt,
                                     bias=zconst[:, 0:1])
                rr = tivp.tile([128, 1], f32, tag="rr")
                nc.vector.reciprocal(rr, rt)            # rsqrt(ms + eps)
                an = tivp.tile([128, 1], f32, tag="an")
                nc.vector.tensor_tensor(out=an, in0=amax, in1=rr,
                                        op=mybir.AluOpType.mult)
                anc = tivp.tile([128, 1], f32, tag="anc")
                nc.vector.tensor_scalar(anc, an, 1e-5, None, mybir.AluOpType.max)
                sr = tivp.tile([128, 1], f32, tag="sr")
                nc.vector.reciprocal(sr, anc)
                s = tivp.tile([128, 1], f32, tag="s")
                nc.vector.tensor_scalar(s, sr, 127.0, None, mybir.AluOpType.mult)
                cq = tivp.tile([128, 1], f32, tag="cq")
                nc.vector.tensor_tensor(out=cq, in0=s, in1=rr,
                                        op=mybir.AluOpType.mult)
                inv_s = tivp.tile([128, 1], f32, tag="invs")
                nc.vector.tensor_scalar(inv_s, anc, 1.0 / 127.0, None,
                                        mybir.AluOpType.mult)
                tinv = tivp.tile([128, 1], f32, tag="tinv")
                nc.vector.tensor_tensor(out=tinv, in0=inv_s, in1=inv_ws,
                                        op=mybir.AluOpType.mult)
                # quantize: q = rint(x * cq)  (|q| <= 127, exact in bf16)
                t1 = qfp.tile([128, DIN], f32, tag="qf")
                nc.scalar.activation(t1, xt, mybir.ActivationFunctionType.Identity,
                                     bias=mconst[:, 0:1], scale=cq[:, 0:1])
                qbf = bfp.tile([128, DIN], bf16, tag="bf")
                nc.vector.tensor_scalar(qbf, t1, MROUND, None,
                                        mybir.AluOpType.subtract)
                # transpose q -> qT [k, tok]
                qTt = qTp.tile([128, KT, 128], bf16, tag="qT")
                for k in range(KT):
                    ps = pst.tile([128, 128], bf16, tag="pst")
                    nc.tensor.transpose(ps, qbf[:, k * 128:(k + 1) * 128], ident)
                    nc.vector.tensor_copy(qTt[:, k, :], ps)
                # matmul + epilogue
                ot = outp.tile([128, O_LOC], f32, tag="out")
                for oc in range(OC):
                    pm = psm.tile([128, 512], f32, tag="psm")
                    for k in range(KT):
                        nc.tensor.matmul(pm, lhsT=qTt[:, k, :],
                                         rhs=wT[:, k, oc * 512:(oc + 1) * 512],
                                         start=(k == 0), stop=(k == KT - 1))
                    nc.scalar.activation(ot[:, oc * 512:(oc + 1) * 512], pm,
                                         mybir.ActivationFunctionType.Copy,
                                         scale=tinv[:, 0:1])
                nc.sync.dma_start(out=out_d[tb * 128:(tb + 1) * 128, :], in_=ot)

    nc.compile()
    return nc


_NC_CACHE = None


def kernel(x: np.ndarray, weight: np.ndarray) -> np.ndarray:
    global _NC_CACHE
    from concourse.bass_utils import run_bass_kernel_spmd

    x = np.ascontiguousarray(np.asarray(x, dtype=np.float32))
    weight = np.ascontiguousarray(np.asarray(weight, dtype=np.float32))
    xf = x.reshape(NTOK, DIN)

    if _NC_CACHE is None:
        _NC_CACHE = build_nc()
    nc = _NC_CACHE

    in_maps = []
    for cid in range(NCORES):
        tr, oc = divmod(cid, C_OUT)
        in_maps.append({
            "x": np.ascontiguousarray(xf[tr * TOK_LOC:(tr + 1) * TOK_LOC]),
            "w": np.ascontiguousarray(weight[oc * O_LOC:(oc + 1) * O_LOC]),
        })

    res = run_bass_kernel_spmd(nc, in_maps, core_ids=list(range(NCORES)))

    out = np.empty((NTOK, DOUT), dtype=np.float32)
    for cid in range(NCORES):
        tr, oc = divmod(cid, C_OUT)
        out[tr * TOK_LOC:(tr + 1) * TOK_LOC,
            oc * O_LOC:(oc + 1) * O_LOC] = res.results[cid]["out"]
    return out.reshape(B, S, DOUT)


if __name__ == "__main__":
    xs = np.random.randn(B, S, DIN).astype(np.float32)
    ws = np.random.randn(DOUT, DIN).astype(np.float32) * 0.01
    o = kernel(x=xs, weight=ws)
    print("kernel ran, out shape", o.shape)
